# revision 14
# baseline (speedup 1.0000x reference)
"""Trainium2 Bass kernel for int8 GEMM + fp32 bias (linear_a8_w8_bfp32_ofp32).

Computes out = (x_int8 @ weight_int8.T).astype(f32) + bias  for
x [8192, 4096] int8, weight [4096, 4096] int8, bias [4096] f32.

Strategy: column-parallel tensor parallelism over 8 NeuronCores — each core
gets all of x (replicated) and a 512-column slice of weight/bias, and
computes its [8192, 512] output slice.

The PE array has no int8 matmul mode (TRN2/cayman dropped UINT8), but
int8 values are exactly representable in bf16, bf16 x bf16 products
(<= 127*127) are exact, and PSUM accumulates in fp32 where every partial
sum of this data stays far below 2^24 — so a bf16 matmul reproduces the
int32-accumulated reference bit-exactly.  The host pre-casts x (and the
k16-31 half of w) to bf16 in DRAM: ring DMAs are write-side limited so
this costs no ring time, and it lets tensors ride the non-casting HWDGE
queues.  bf16 N=512 streaming (215.8 ns/MM) is the PE floor.

Measured trace model (ntff):
- exec_time = last_useful - first_useful; first_useful ~5.9us (framework
  preamble excluded), last_useful = end of walrus's fixed epilogue that
  zeroes all 255 semaphores one EVENT_SEMAPHORE each, split across the 5
  engines (Tensor slowest, 53 sems at ~115ns apiece when the HAM has
  re-throttled the idle core).  Every ns the real work finishes earlier
  shifts the epilogue 1:1; dummy PE matmuls during the store-drain window
  keep the clock up so the epilogue runs at the warm rate.
- SWDGE ring: first chunk data ~4.7us after the first descriptor, then
  ~0.69us per 256KB of SBUF writes; descriptor-gen ~0.67us on GpSimd.
- HWDGE queues: sync ~53GB/s, scalar ~35GB/s (+~1.3us latency), vector
  assumed ~sync.  No in-DMA dtype casting.
- HAM: first ~7-10 MMs run at half clock (427ns), then full speed.
- DVE int8->bf16 cast of [128,512]: ~0.35us; scalar ACTIVATE: ~0.72us.

Ramp: m-tiles 0 and 1 run as two interleaved PSUM chains, halving
k-consumption to 432ns/k-tile.  The ring carries w k0-15 raw int8 (each
4k-tile chunk ahead of its x0 partner so the k0 cast bounds T0), then
w k16-31 as bf16 (no cast needed), then the m-loop x tiles.  x1 rides
the three idle HWDGE queues (sync k0-7, vector k8-19, scalar k20-31
then bias).  DVE casts w k0-15 (k%4!=3), scalar every 4th.  Every
arrival beats its dual-chain consumption slot by >=0.7us.  Warmup is
13 garbage MMs ending right at T0 (~13.3us).

Tail: last m-tile splits into 256/160/96-wide chains (epilogues hide
behind the next chain); stores go sync/vector/sync, regular m-tile
stores alternate sync (odd) / vector (even) so both queues are nearly
drained when the tail lands.  After the last chain, 12 dummy matmuls
keep the PE busy through the store drain (they finish before Sync
reaches the final barrier, so they cost nothing).
"""

import ml_dtypes
import numpy as np

import concourse.mybir as mybir
import concourse.tile as tile
from concourse import bacc
from concourse.bass_utils import run_bass_kernel_spmd

P = 128
N_CORES = 8
KRAW = 16  # w k-tiles arriving raw int8 (cast on DVE/scalar); rest bf16

# Set by a test harness to capture timing/trace info; harmless defaults.
TRACE = False
TRACE_KWARGS = {}
LAST_RESULT = None


def build_program(MT, KT, NLOC, x_bufs=4, o_bufs=3, psum_bufs=3, warmup_mms=13,
                  drain_mms=12):
    """Bass/Tile program for one core: out[MT*128, NLOC] = xT.T @ wT + bias.

    DRAM layouts (host pre-arranged, all contiguous per SBUF partition):
      x_tiles   [MT, P, KT, P]    bf16  x_tiles[mt, ki, kt, mi] = x[mt*P+mi, kt*P+ki]
      w_raw     [P, KRAW, NLOC]   int8  w_raw[ki, kt, n] = weight[n, kt*P+ki]
      w_b16     [P, KT-KRAW, NLOC] bf16 same for kt >= KRAW
      bias_bcast[P, NLOC]         f32   bias replicated across partitions
      out_tiles [MT, P, NLOC]     f32   out_tiles[mt, mi, n] = out[mt*P+mi, n]
    """
    nc = bacc.Bacc()
    x_d = nc.declare_dram_parameter(
        "x_tiles", [MT, P, KT, P], mybir.dt.bfloat16, isOutput=False
    )
    wr_d = nc.declare_dram_parameter(
        "w_raw", [P, KRAW, NLOC], mybir.dt.int8, isOutput=False
    )
    wb_d = nc.declare_dram_parameter(
        "w_b16", [P, KT - KRAW, NLOC], mybir.dt.bfloat16, isOutput=False
    )
    b_d = nc.declare_dram_parameter(
        "bias_bcast", [P, NLOC], mybir.dt.float32, isOutput=False
    )
    o_d = nc.declare_dram_parameter(
        "out_tiles", [MT, P, NLOC], mybir.dt.float32, isOutput=True
    )

    with tile.TileContext(nc) as tc:
        with (
            tc.tile_pool(name="wqpool", bufs=1) as wqpool,
            tc.tile_pool(name="wkpool", bufs=1) as wkpool,
            tc.tile_pool(name="cpool", bufs=1) as cpool,
            tc.tile_pool(name="x01pool", bufs=1) as x01pool,
            tc.tile_pool(name="xpool", bufs=x_bufs) as xpool,
            tc.tile_pool(name="opool", bufs=o_bufs) as opool,
            tc.tile_pool(name="otail", bufs=2) as otail,
            tc.tile_pool(name="psum", bufs=psum_bufs, space="PSUM") as psum_pool,
            tc.tile_pool(name="psab", bufs=1, space="PSUM") as psab_pool,
            tc.tile_pool(name="pst", bufs=1, space="PSUM") as pst_pool,
            tc.tile_pool(name="warm", bufs=1) as warm_pool,
        ):
            # PE warmup: garbage matmuls un-throttle the HAM (~7-10 MMs at
            # half clock, then full speed) and end right as the first real
            # k-tile is ready.  memset on DVE so the gpsimd queue starts
            # ring descriptor-gen immediately.  Accumulates into chain A's
            # PSUM bank (never read; chain A's start=True MM resets it).
            ps_a = psab_pool.tile([P, NLOC], mybir.dt.float32, tag="psA", name="psA")
            ps_b = psab_pool.tile([P, NLOC], mybir.dt.float32, tag="psB", name="psB")
            wu = warm_pool.tile([P, NLOC], mybir.dt.bfloat16)
            nc.vector.memset(wu[:], 0.0)
            if warmup_mms:
                for i in range(warmup_mms):
                    nc.tensor.matmul(
                        ps_a[:],
                        wu[:, :P],
                        wu[:],
                        start=(i == 0),
                        stop=(i == warmup_mms - 1),
                    )

            # x1 rides the otherwise-idle HWDGE queues.
            x1_sb = {}

            def emit_x1(eng, k0, k1):
                t = x01pool.tile(
                    [P, k1 - k0, P], mybir.dt.bfloat16,
                    tag=f"x1c{k0}", name=f"x1c{k0}",
                )
                eng.dma_start(out=t[:], in_=x_d[1, :, k0:k1, :])
                x1_sb[k0] = (t, k0, k1)

            for j in range(8):
                emit_x1(nc.sync if j % 2 == 0 else nc.scalar, 4 * j, 4 * j + 4)
            # bias queues on sync behind its x1 blocks; needed only at ~27us.
            b_sb = cpool.tile([P, NLOC], mybir.dt.float32)
            nc.sync.dma_start(out=b_sb[:], in_=b_d[:])

            # gpsimd SWDGE ring, strict FIFO: raw w chunk j leads its x0
            # partner so the k0 cast bounds T0, then the bf16 w tail, then
            # the m-loop x tiles.
            x0_sb = {}
            wq_raw = {}
            wb_sb = {}
            for j in range(4):
                t = wqpool.tile(
                    [P, 4, NLOC], mybir.dt.int8, tag=f"wq{j}", name=f"wq{j}"
                )
                nc.gpsimd.dma_start(out=t[:], in_=wr_d[:, 4 * j : 4 * j + 4, :])
                wq_raw[j] = t
                xt = x01pool.tile(
                    [P, 8, P], mybir.dt.bfloat16, tag=f"x0c{j}", name=f"x0c{j}"
                )
                nc.gpsimd.dma_start(out=xt[:], in_=x_d[0, :, 8 * j : 8 * j + 8, :])
                x0_sb[8 * j] = (xt, 8 * j, 8 * j + 8)
            for j in range(4, 8):
                t = wkpool.tile(
                    [P, 4, NLOC], mybir.dt.bfloat16, tag=f"wb{j}", name=f"wb{j}"
                )
                nc.gpsimd.dma_start(
                    out=t[:], in_=wb_d[:, 4 * j - KRAW : 4 * j + 4 - KRAW, :]
                )
                wb_sb[j] = t

            # w k0-15 casts in ring-arrival (= k) order; scalar every 4th.
            wk = {}
            for k in range(KRAW):
                wk[k] = wkpool.tile(
                    [P, NLOC], mybir.dt.bfloat16, tag=f"wk{k}", name=f"wk{k}"
                )
                src = wq_raw[k // 4][:, k % 4, :]
                if k % 4 == 3:
                    nc.scalar.copy(wk[k][:], src)
                else:
                    nc.vector.tensor_copy(wk[k][:], src)

            def w_slice(kt):
                if kt < KRAW:
                    return wk[kt][:]
                return wb_sb[kt // 4][:, kt % 4, :]

            def x01_slice(sb, kt):
                for t, k0, k1 in sb.values():
                    if k0 <= kt < k1:
                        return t[:, kt - k0, :]
                raise KeyError(kt)

            # --- m-tiles 0+1: interleaved dual PSUM chains ---------------
            for kt in range(KT):
                nc.tensor.matmul(
                    ps_a[:], x01_slice(x0_sb, kt), w_slice(kt),
                    start=(kt == 0), stop=(kt == KT - 1),
                )
                nc.tensor.matmul(
                    ps_b[:], x01_slice(x1_sb, kt), w_slice(kt),
                    start=(kt == 0), stop=(kt == KT - 1),
                )
            for mt, ps in ((0, ps_a), (1, ps_b)):
                o_sb = opool.tile([P, NLOC], mybir.dt.float32)
                nc.vector.tensor_add(o_sb[:], ps[:], b_sb[:])
                nc.sync.dma_start(out=o_d[mt], in_=o_sb[:])

            # --- main m-tile loop ----------------------------------------
            for mt in range(2, MT):
                x_sb = xpool.tile([P, KT, P], mybir.dt.bfloat16)
                nc.gpsimd.dma_start(out=x_sb[:], in_=x_d[mt])
                if mt < MT - 1:
                    ps = psum_pool.tile([P, NLOC], mybir.dt.float32)
                    for kt in range(KT):
                        nc.tensor.matmul(
                            ps[:],
                            x_sb[:, kt, :],
                            w_slice(kt),
                            start=(kt == 0),
                            stop=(kt == KT - 1),
                        )
                    o_sb = opool.tile([P, NLOC], mybir.dt.float32)
                    nc.vector.tensor_add(o_sb[:], ps[:], b_sb[:])
                    nc.sync.dma_start(out=o_d[mt], in_=o_sb[:])
                else:
                    # last m-tile: progressively narrower chains so each
                    # epilogue hides behind the next chain's matmuls, with
                    # the final 96-wide store on the nearly-drained sync
                    # queue.
                    spans = [(0, 256), (256, 416), (416, NLOC)]
                    engs = [nc.scalar, nc.sync, nc.sync]
                    for h, (n0, n1) in enumerate(spans):
                        ph = pst_pool.tile(
                            [P, n1 - n0], mybir.dt.float32,
                            tag=f"pst{h}", name=f"pst{h}",
                        )
                        for kt in range(KT):
                            nc.tensor.matmul(
                                ph[:],
                                x_sb[:, kt, :],
                                w_slice(kt)[:, n0:n1],
                                start=(kt == 0),
                                stop=(kt == KT - 1),
                            )
                        o_h = otail.tile(
                            [P, n1 - n0], mybir.dt.float32,
                            tag=f"ot{h}", name=f"ot{h}",
                        )
                        nc.vector.tensor_add(o_h[:], ph[:], b_sb[:, n0:n1])
                        engs[h].dma_start(out=o_d[mt, :, n0:n1], in_=o_h[:])

            # Dummy matmuls keep the HAM clock up through the store drain
            # so walrus's semaphore-zero epilogue runs at the warm rate.
            # They end before Sync reaches the final barrier: free.
            for i in range(drain_mms):
                nc.tensor.matmul(
                    ps_a[:],
                    wu[:, :P],
                    wu[:],
                    start=(i == 0),
                    stop=(i == drain_mms - 1),
                )
    nc.compile()
    return nc


def run(x, weight, fake_bias):
    global LAST_RESULT
    M, K = x.shape
    N = weight.shape[0]
    assert M % P == 0 and K % P == 0 and N % (N_CORES * P) == 0
    MT, KT, NLOC = M // P, K // P, N // N_CORES

    xb = np.asarray(x).astype(np.int8)
    x_tiles = np.ascontiguousarray(
        xb.reshape(MT, P, KT, P).transpose(0, 3, 2, 1)
    ).astype(ml_dtypes.bfloat16)
    wb = np.asarray(weight).astype(np.int8)
    bias = np.asarray(fake_bias).astype(np.float32)

    in_maps = []
    for c in range(N_CORES):
        w_loc = wb[c * NLOC : (c + 1) * NLOC, :]  # [NLOC, K]
        w_tiles = np.ascontiguousarray(
            w_loc.T.reshape(KT, P, NLOC).transpose(1, 0, 2)
        )
        w_raw = np.ascontiguousarray(w_tiles[:, :KRAW, :])
        w_b16 = np.ascontiguousarray(w_tiles[:, KRAW:, :]).astype(
            ml_dtypes.bfloat16
        )
        b_loc = np.ascontiguousarray(
            np.broadcast_to(bias[None, c * NLOC : (c + 1) * NLOC], (P, NLOC))
        )
        in_maps.append(
            {
                "x_tiles": x_tiles,
                "w_raw": w_raw,
                "w_b16": w_b16,
                "bias_bcast": b_loc,
            }
        )

    nc = build_program(MT, KT, NLOC)
    res = run_bass_kernel_spmd(
        nc, in_maps, list(range(N_CORES)), trace=TRACE, **TRACE_KWARGS
    )
    LAST_RESULT = res

    outs = [r["out_tiles"].reshape(M, NLOC) for r in res.results]
    return np.concatenate(outs, axis=1).astype(np.float32)


def kernel(x, weight, fake_bias):
    return run(x, weight, fake_bias)


# revision 15
# speedup vs baseline: 1.0107x; 1.0107x over previous
"""Trainium2 Bass kernel for int8 GEMM + fp32 bias (linear_a8_w8_bfp32_ofp32).

Computes out = (x_int8 @ weight_int8.T).astype(f32) + bias  for
x [8192, 4096] int8, weight [4096, 4096] int8, bias [4096] f32.

Strategy: column-parallel tensor parallelism over 8 NeuronCores — each core
gets all of x (replicated) and a 512-column slice of weight/bias, and
computes its [8192, 512] output slice.

The PE array has no int8 matmul mode (TRN2/cayman dropped UINT8), but
int8 values are exactly representable in bf16, bf16 x bf16 products
(<= 127*127) are exact, and PSUM accumulates in fp32 where every partial
sum of this data stays far below 2^24 — so a bf16 matmul reproduces the
int32-accumulated reference bit-exactly.  bf16 N=512 streaming
(215.8 ns/MM) is the PE floor: 2048 MM-equivalents = 442us/core.

Measured machine model (from ntff traces):
- exec_time = last_useful - first_useful; first_useful ~6us (framework
  preamble excluded), last_useful = end of walrus's fixed epilogue that
  zeroes all 255 semaphores one EVENT_SEMAPHORE each, split across the
  5 engines.  The rates are intrinsic sequencer speeds (Tensor 115ns/op
  x 53 sems — measured identical warm or throttled), so the epilogue is
  a fixed ~8.9us after the last DMA drains; every ns the real work
  finishes earlier shifts it 1:1.
- SWDGE ring: first chunk data ~12.55us (descr-gen from ~7.85 at
  0.67us/chunk, ~4.7us pipe), then ~0.69us per 256KB of SBUF writes.
- HWDGE queues (sync/scalar only): stores ~53/~35 GB/s, but multi-us
  COLD-start latency — keep a queue warm before relying on it late.
- HAM: evaluates ~3.4us windows; a PE idle gap >~2us triggers one
  half-clock window.  First ~7-10 warmup MMs run at 427ns, then 216.
- DVE int8->bf16 cast of [128,512]: ~0.43us; scalar ACTIVATE: ~0.72us.

Ramp: m-tiles 0 and 1 run as two interleaved PSUM chains (A-k,B-k,...),
halving k-consumption to 432ns/k-tile so the ring + casts keep pace.
Ring order w0, x0c0, x1c0, w1, x0c1, w2, x1c1, x0c2, w3, x0c3, w4,
x1c2, w5, w6, w7, x1c3 puts every arrival >=0.4us ahead of its
consumption slot (w raw int8, cast by DVE k%4!=3 / scalar k%4==3 in
arrival order; x via casting DMAs).  Warmup is 14 garbage MMs ending
right at the first real MM (~13.5us) with no HAM-rethrottling gap.

Tail: last m-tile splits into 256/160/96-wide chains (epilogues hide
behind the next chain's matmuls); chain stores go sync/scalar/sync and
every 8th m-tile store rides scalar to keep that queue warm, so the
final 48KB store drains ~1us after the last bias-add.
"""

import numpy as np

import concourse.mybir as mybir
import concourse.tile as tile
from concourse import bacc
from concourse.bass_utils import run_bass_kernel_spmd

P = 128
N_CORES = 8

# Set by a test harness to capture timing/trace info; harmless defaults.
TRACE = False
TRACE_KWARGS = {}
LAST_RESULT = None


def build_program(MT, KT, NLOC, x_bufs=4, o_bufs=3, psum_bufs=3, warmup_mms=14):
    """Bass/Tile program for one core: out[MT*128, NLOC] = xT.T @ wT + bias.

    DRAM layouts (host pre-arranged, all contiguous per SBUF partition):
      x_tiles   [MT, P, KT, P]  int8   x_tiles[mt, ki, kt, mi] = x[mt*P+mi, kt*P+ki]
      w_tiles   [P, KT, NLOC]   int8   w_tiles[ki, kt, n] = weight[n, kt*P+ki]
      bias_bcast[P, NLOC]       f32    bias replicated across partitions
      out_tiles [MT, P, NLOC]   f32    out_tiles[mt, mi, n] = out[mt*P+mi, n]
    """
    nc = bacc.Bacc()
    x_d = nc.declare_dram_parameter(
        "x_tiles", [MT, P, KT, P], mybir.dt.int8, isOutput=False
    )
    w_d = nc.declare_dram_parameter(
        "w_tiles", [P, KT, NLOC], mybir.dt.int8, isOutput=False
    )
    b_d = nc.declare_dram_parameter(
        "bias_bcast", [P, NLOC], mybir.dt.float32, isOutput=False
    )
    o_d = nc.declare_dram_parameter(
        "out_tiles", [MT, P, NLOC], mybir.dt.float32, isOutput=True
    )

    with tile.TileContext(nc) as tc:
        with (
            tc.tile_pool(name="wqpool", bufs=1) as wqpool,
            tc.tile_pool(name="wkpool", bufs=1) as wkpool,
            tc.tile_pool(name="cpool", bufs=1) as cpool,
            tc.tile_pool(name="x01pool", bufs=1) as x01pool,
            tc.tile_pool(name="xpool", bufs=x_bufs) as xpool,
            tc.tile_pool(name="opool", bufs=o_bufs) as opool,
            tc.tile_pool(name="otail", bufs=2) as otail,
            tc.tile_pool(name="psum", bufs=psum_bufs, space="PSUM") as psum_pool,
            tc.tile_pool(name="psab", bufs=1, space="PSUM") as psab_pool,
            tc.tile_pool(name="pst", bufs=1, space="PSUM") as pst_pool,
            tc.tile_pool(name="warm", bufs=1) as warm_pool,
        ):
            # PE warmup: garbage matmuls un-throttle the HAM and end right
            # as the first real k-tile is ready, with no re-throttling
            # idle gap.  memset on DVE so the gpsimd queue starts ring
            # descriptor-gen immediately.  Accumulates into chain A's
            # PSUM bank (never read; chain A's start=True MM resets it).
            ps_a = psab_pool.tile([P, NLOC], mybir.dt.float32, tag="psA", name="psA")
            ps_b = psab_pool.tile([P, NLOC], mybir.dt.float32, tag="psB", name="psB")
            wu = warm_pool.tile([P, NLOC], mybir.dt.bfloat16)
            nc.vector.memset(wu[:], 0.0)
            if warmup_mms:
                for i in range(warmup_mms):
                    nc.tensor.matmul(
                        ps_a[:],
                        wu[:, :P],
                        wu[:],
                        start=(i == 0),
                        stop=(i == warmup_mms - 1),
                    )

            # gpsimd SWDGE ring, strict FIFO.  Every chunk lands >=0.4us
            # before its dual-chain consumption slot.
            x0_sb = {}
            x1_sb = {}
            wq_raw = {}

            def emit_x(m, k0, k1):
                t = x01pool.tile(
                    [P, k1 - k0, P], mybir.dt.bfloat16,
                    tag=f"x{m}c{k0}", name=f"x{m}c{k0}",
                )
                nc.gpsimd.dma_start(out=t[:], in_=x_d[m, :, k0:k1, :])
                (x0_sb if m == 0 else x1_sb)[k0] = (t, k0, k1)

            def emit_w(j):
                t = wqpool.tile(
                    [P, 4, NLOC], mybir.dt.int8, tag=f"wq{j}", name=f"wq{j}"
                )
                nc.gpsimd.dma_start(out=t[:], in_=w_d[:, 4 * j : 4 * j + 4, :])
                wq_raw[j] = t

            emit_w(0)
            emit_x(0, 0, 8)
            emit_x(1, 0, 8)
            emit_w(1)
            emit_x(0, 8, 16)
            emit_w(2)
            emit_x(1, 8, 16)
            emit_x(0, 16, 24)
            emit_w(3)
            emit_x(0, 24, 32)
            emit_w(4)
            emit_x(1, 16, 24)
            emit_w(5)
            emit_w(6)
            emit_w(7)
            emit_x(1, 24, 32)

            # bias on sync; data needed only at ~27.5us so the cold-start
            # latency is irrelevant (and it warms the sync queue).
            b_sb = cpool.tile([P, NLOC], mybir.dt.float32)
            nc.sync.dma_start(out=b_sb[:], in_=b_d[:])

            # w casts in ring-arrival (= k) order; scalar takes every 4th.
            wk = {}
            for k in range(KT):
                wk[k] = wkpool.tile(
                    [P, NLOC], mybir.dt.bfloat16, tag=f"wk{k}", name=f"wk{k}"
                )
                src = wq_raw[k // 4][:, k % 4, :]
                if k % 4 == 3:
                    nc.scalar.copy(wk[k][:], src)
                else:
                    nc.vector.tensor_copy(wk[k][:], src)

            def x01_slice(sb, kt):
                for t, k0, k1 in sb.values():
                    if k0 <= kt < k1:
                        return t[:, kt - k0, :]
                raise KeyError(kt)

            # --- m-tiles 0+1: interleaved dual PSUM chains ---------------
            for kt in range(KT):
                nc.tensor.matmul(
                    ps_a[:], x01_slice(x0_sb, kt), wk[kt][:],
                    start=(kt == 0), stop=(kt == KT - 1),
                )
                nc.tensor.matmul(
                    ps_b[:], x01_slice(x1_sb, kt), wk[kt][:],
                    start=(kt == 0), stop=(kt == KT - 1),
                )
            for mt, ps in ((0, ps_a), (1, ps_b)):
                o_sb = opool.tile([P, NLOC], mybir.dt.float32)
                nc.vector.tensor_add(o_sb[:], ps[:], b_sb[:])
                nc.sync.dma_start(out=o_d[mt], in_=o_sb[:])

            # --- main m-tile loop ----------------------------------------
            for mt in range(2, MT):
                x_sb = xpool.tile([P, KT, P], mybir.dt.bfloat16)
                nc.gpsimd.dma_start(out=x_sb[:], in_=x_d[mt])
                if mt < MT - 1:
                    ps = psum_pool.tile([P, NLOC], mybir.dt.float32)
                    for kt in range(KT):
                        nc.tensor.matmul(
                            ps[:],
                            x_sb[:, kt, :],
                            wk[kt][:],
                            start=(kt == 0),
                            stop=(kt == KT - 1),
                        )
                    o_sb = opool.tile([P, NLOC], mybir.dt.float32)
                    nc.vector.tensor_add(o_sb[:], ps[:], b_sb[:])
                    # every 8th store rides scalar: keeps that queue warm
                    # for the tail and relieves the sync backlog.
                    (nc.scalar if mt % 8 == 5 else nc.sync).dma_start(
                        out=o_d[mt], in_=o_sb[:]
                    )
                else:
                    # last m-tile: progressively narrower chains so each
                    # epilogue hides behind the next chain's matmuls, with
                    # the final 96-wide store on the nearly-drained sync
                    # queue.
                    spans = [(0, 256), (256, 416), (416, NLOC)]
                    engs = [nc.sync, nc.scalar, nc.sync]
                    for h, (n0, n1) in enumerate(spans):
                        ph = pst_pool.tile(
                            [P, n1 - n0], mybir.dt.float32,
                            tag=f"pst{h}", name=f"pst{h}",
                        )
                        for kt in range(KT):
                            nc.tensor.matmul(
                                ph[:],
                                x_sb[:, kt, :],
                                wk[kt][:, n0:n1],
                                start=(kt == 0),
                                stop=(kt == KT - 1),
                            )
                        o_h = otail.tile(
                            [P, n1 - n0], mybir.dt.float32,
                            tag=f"ot{h}", name=f"ot{h}",
                        )
                        nc.vector.tensor_add(o_h[:], ph[:], b_sb[:, n0:n1])
                        engs[h].dma_start(out=o_d[mt, :, n0:n1], in_=o_h[:])
    nc.compile()
    return nc


def run(x, weight, fake_bias):
    global LAST_RESULT
    M, K = x.shape
    N = weight.shape[0]
    assert M % P == 0 and K % P == 0 and N % (N_CORES * P) == 0
    MT, KT, NLOC = M // P, K // P, N // N_CORES

    xb = np.asarray(x).astype(np.int8)
    x_tiles = np.ascontiguousarray(xb.reshape(MT, P, KT, P).transpose(0, 3, 2, 1))
    wb = np.asarray(weight).astype(np.int8)
    bias = np.asarray(fake_bias).astype(np.float32)

    in_maps = []
    for c in range(N_CORES):
        w_loc = wb[c * NLOC : (c + 1) * NLOC, :]  # [NLOC, K]
        w_tiles = np.ascontiguousarray(
            w_loc.T.reshape(KT, P, NLOC).transpose(1, 0, 2)
        )
        b_loc = np.ascontiguousarray(
            np.broadcast_to(bias[None, c * NLOC : (c + 1) * NLOC], (P, NLOC))
        )
        in_maps.append(
            {"x_tiles": x_tiles, "w_tiles": w_tiles, "bias_bcast": b_loc}
        )

    nc = build_program(MT, KT, NLOC)
    res = run_bass_kernel_spmd(
        nc, in_maps, list(range(N_CORES)), trace=TRACE, **TRACE_KWARGS
    )
    LAST_RESULT = res

    outs = [r["out_tiles"].reshape(M, NLOC) for r in res.results]
    return np.concatenate(outs, axis=1).astype(np.float32)


def kernel(x, weight, fake_bias):
    return run(x, weight, fake_bias)


# revision 17
# speedup vs baseline: 1.0119x; 1.0012x over previous
"""Trainium2 Bass kernel for int8 GEMM + fp32 bias (linear_a8_w8_bfp32_ofp32).

Computes out = (x_int8 @ weight_int8.T).astype(f32) + bias  for
x [8192, 4096] int8, weight [4096, 4096] int8, bias [4096] f32.

Strategy: column-parallel tensor parallelism over 8 NeuronCores — each core
gets all of x (replicated) and a 512-column slice of weight/bias, and
computes its [8192, 512] output slice.

The PE array has no int8 matmul mode (TRN2/cayman dropped UINT8), but
int8 values are exactly representable in bf16, bf16 x bf16 products
(<= 127*127) are exact, and PSUM accumulates in fp32 where every partial
sum of this data stays far below 2^24 — so a bf16 matmul reproduces the
int32-accumulated reference bit-exactly.  bf16 N=512 streaming
(215.8 ns/MM) is the PE floor: 2048 MM-equivalents = 442us/core.

Measured machine model (from ntff traces):
- exec_time = last_useful - first_useful; first_useful ~6us (framework
  preamble excluded), last_useful = end of walrus's fixed epilogue that
  zeroes all 255 semaphores one EVENT_SEMAPHORE each, split across the
  5 engines.  The rates are intrinsic sequencer speeds (Tensor 115ns/op
  x 53 sems — measured identical warm or throttled), so the epilogue is
  a fixed ~8.9us after the last DMA drains; every ns the real work
  finishes earlier shifts it 1:1.
- SWDGE ring: first chunk data ~12.55us (descr-gen from ~7.85 at
  0.67us/chunk, ~4.7us pipe), then ~0.69us per 256KB of SBUF writes.
- HWDGE queues (sync/scalar only): stores ~53/~35 GB/s, but multi-us
  COLD-start latency — keep a queue warm before relying on it late.
- HAM: evaluates ~3.4us windows; a PE idle gap >~2us triggers one
  half-clock window.  First ~7-10 warmup MMs run at 427ns, then 216.
- DVE int8->bf16 cast of [128,512]: ~0.43us; scalar ACTIVATE: ~0.72us.

Ramp: m-tiles 0 and 1 run as two interleaved PSUM chains (A-k,B-k,...),
halving k-consumption to 432ns/k-tile so the ring + casts keep pace.
Ring order w0, x0c0, x1c0, w1, x0c1, w2, x1c1, x0c2, w3, x0c3, w4,
x1c2, w5, w6, w7, x1c3 puts every arrival >=0.4us ahead of its
consumption slot (w raw int8, cast by DVE k%4!=3 / scalar k%4==3 in
arrival order; x via casting DMAs).  Warmup is 14 garbage MMs ending
right at the first real MM (~13.5us) with no HAM-rethrottling gap.

Tail: last m-tile splits into 256/160/96-wide chains (epilogues hide
behind the next chain's matmuls); chain stores go sync/scalar/sync and
every 8th m-tile store rides scalar to keep that queue warm, so the
final 48KB store drains ~1us after the last bias-add.
"""

import numpy as np

import concourse.mybir as mybir
import concourse.tile as tile
from concourse import bacc
from concourse.bass_utils import run_bass_kernel_spmd

P = 128
N_CORES = 8

# Set by a test harness to capture timing/trace info; harmless defaults.
TRACE = False
TRACE_KWARGS = {}
LAST_RESULT = None


def build_program(MT, KT, NLOC, x_bufs=4, o_bufs=3, psum_bufs=3, warmup_mms=12):
    """Bass/Tile program for one core: out[MT*128, NLOC] = xT.T @ wT + bias.

    DRAM layouts (host pre-arranged, all contiguous per SBUF partition):
      x_tiles   [MT, P, KT, P]  int8   x_tiles[mt, ki, kt, mi] = x[mt*P+mi, kt*P+ki]
      w_tiles   [P, KT, NLOC]   int8   w_tiles[ki, kt, n] = weight[n, kt*P+ki]
      bias_bcast[P, NLOC]       f32    bias replicated across partitions
      out_tiles [MT, P, NLOC]   f32    out_tiles[mt, mi, n] = out[mt*P+mi, n]
    """
    nc = bacc.Bacc()
    x_d = nc.declare_dram_parameter(
        "x_tiles", [MT, P, KT, P], mybir.dt.int8, isOutput=False
    )
    w_d = nc.declare_dram_parameter(
        "w_tiles", [P, KT, NLOC], mybir.dt.int8, isOutput=False
    )
    b_d = nc.declare_dram_parameter(
        "bias_bcast", [P, NLOC], mybir.dt.float32, isOutput=False
    )
    o_d = nc.declare_dram_parameter(
        "out_tiles", [MT, P, NLOC], mybir.dt.float32, isOutput=True
    )

    with tile.TileContext(nc) as tc:
        with (
            tc.tile_pool(name="wqpool", bufs=1) as wqpool,
            tc.tile_pool(name="wkpool", bufs=1) as wkpool,
            tc.tile_pool(name="cpool", bufs=1) as cpool,
            tc.tile_pool(name="x01pool", bufs=1) as x01pool,
            tc.tile_pool(name="xpool", bufs=x_bufs) as xpool,
            tc.tile_pool(name="opool", bufs=o_bufs) as opool,
            tc.tile_pool(name="otail", bufs=2) as otail,
            tc.tile_pool(name="psum", bufs=psum_bufs, space="PSUM") as psum_pool,
            tc.tile_pool(name="psab", bufs=1, space="PSUM") as psab_pool,
            tc.tile_pool(name="pst", bufs=1, space="PSUM") as pst_pool,
            tc.tile_pool(name="warm", bufs=1) as warm_pool,
        ):
            # PE warmup: garbage matmuls un-throttle the HAM and end right
            # as the first real k-tile is ready, with no re-throttling
            # idle gap.  memset on DVE so the gpsimd queue starts ring
            # descriptor-gen immediately.  Accumulates into chain A's
            # PSUM bank (never read; chain A's start=True MM resets it).
            ps_a = psab_pool.tile([P, NLOC], mybir.dt.float32, tag="psA", name="psA")
            ps_b = psab_pool.tile([P, NLOC], mybir.dt.float32, tag="psB", name="psB")
            wu = warm_pool.tile([P, NLOC], mybir.dt.bfloat16)
            nc.vector.memset(wu[:], 0.0)
            if warmup_mms:
                for i in range(warmup_mms):
                    nc.tensor.matmul(
                        ps_a[:],
                        wu[:, :P],
                        wu[:],
                        start=(i == 0),
                        stop=(i == warmup_mms - 1),
                    )

            # gpsimd SWDGE ring, strict FIFO.  Every chunk lands >=0.4us
            # before its dual-chain consumption slot.
            x0_sb = {}
            x1_sb = {}
            wq_raw = {}

            def emit_x(m, k0, k1):
                t = x01pool.tile(
                    [P, k1 - k0, P], mybir.dt.bfloat16,
                    tag=f"x{m}c{k0}", name=f"x{m}c{k0}",
                )
                nc.gpsimd.dma_start(out=t[:], in_=x_d[m, :, k0:k1, :])
                (x0_sb if m == 0 else x1_sb)[k0] = (t, k0, k1)

            def emit_w(j):
                t = wqpool.tile(
                    [P, 4, NLOC], mybir.dt.int8, tag=f"wq{j}", name=f"wq{j}"
                )
                nc.gpsimd.dma_start(out=t[:], in_=w_d[:, 4 * j : 4 * j + 4, :])
                wq_raw[j] = t

            emit_w(0)
            emit_x(0, 0, 8)
            emit_x(1, 0, 8)
            emit_w(1)
            emit_x(0, 8, 16)
            emit_w(2)
            emit_x(1, 8, 16)
            emit_x(0, 16, 24)
            emit_w(3)
            emit_x(0, 24, 32)
            emit_w(4)
            emit_x(1, 16, 24)
            emit_w(5)
            emit_w(6)
            emit_w(7)
            emit_x(1, 24, 32)

            # bias on sync; data needed only at ~27.5us so the cold-start
            # latency is irrelevant (and it warms the sync queue).
            b_sb = cpool.tile([P, NLOC], mybir.dt.float32)
            nc.sync.dma_start(out=b_sb[:], in_=b_d[:])

            # w casts in ring-arrival (= k) order; scalar takes every 4th.
            wk = {}
            for k in range(KT):
                wk[k] = wkpool.tile(
                    [P, NLOC], mybir.dt.bfloat16, tag=f"wk{k}", name=f"wk{k}"
                )
                src = wq_raw[k // 4][:, k % 4, :]
                if k % 4 == 3:
                    nc.scalar.copy(wk[k][:], src)
                else:
                    nc.vector.tensor_copy(wk[k][:], src)

            def x01_slice(sb, kt):
                for t, k0, k1 in sb.values():
                    if k0 <= kt < k1:
                        return t[:, kt - k0, :]
                raise KeyError(kt)

            # --- m-tiles 0+1: interleaved dual PSUM chains ---------------
            for kt in range(KT):
                nc.tensor.matmul(
                    ps_a[:], x01_slice(x0_sb, kt), wk[kt][:],
                    start=(kt == 0), stop=(kt == KT - 1),
                )
                nc.tensor.matmul(
                    ps_b[:], x01_slice(x1_sb, kt), wk[kt][:],
                    start=(kt == 0), stop=(kt == KT - 1),
                )
            for mt, ps in ((0, ps_a), (1, ps_b)):
                o_sb = opool.tile([P, NLOC], mybir.dt.float32)
                nc.vector.tensor_add(o_sb[:], ps[:], b_sb[:])
                nc.sync.dma_start(out=o_d[mt], in_=o_sb[:])

            # --- main m-tile loop ----------------------------------------
            for mt in range(2, MT):
                x_sb = xpool.tile([P, KT, P], mybir.dt.bfloat16)
                nc.gpsimd.dma_start(out=x_sb[:], in_=x_d[mt])
                if mt < MT - 1:
                    ps = psum_pool.tile([P, NLOC], mybir.dt.float32)
                    for kt in range(KT):
                        nc.tensor.matmul(
                            ps[:],
                            x_sb[:, kt, :],
                            wk[kt][:],
                            start=(kt == 0),
                            stop=(kt == KT - 1),
                        )
                    o_sb = opool.tile([P, NLOC], mybir.dt.float32)
                    nc.vector.tensor_add(o_sb[:], ps[:], b_sb[:])
                    # every 8th store rides scalar: keeps that queue warm
                    # for the tail and relieves the sync backlog.  m62's
                    # store splits across both queues so each is drained
                    # when the tail chains land.
                    if mt == MT - 2:
                        NH = NLOC // 2
                        nc.sync.dma_start(
                            out=o_d[mt, :, :NH], in_=o_sb[:, :NH]
                        )
                        nc.scalar.dma_start(
                            out=o_d[mt, :, NH:], in_=o_sb[:, NH:]
                        )
                    else:
                        (nc.scalar if mt % 8 == 5 else nc.sync).dma_start(
                            out=o_d[mt], in_=o_sb[:]
                        )
                else:
                    # last m-tile: progressively narrower chains so each
                    # epilogue hides behind the next chain's matmuls, with
                    # the final 96-wide store on the nearly-drained sync
                    # queue.
                    spans = [(0, 256), (256, 416), (416, NLOC)]
                    engs = [nc.sync, nc.scalar, nc.sync]
                    for h, (n0, n1) in enumerate(spans):
                        ph = pst_pool.tile(
                            [P, n1 - n0], mybir.dt.float32,
                            tag=f"pst{h}", name=f"pst{h}",
                        )
                        for kt in range(KT):
                            nc.tensor.matmul(
                                ph[:],
                                x_sb[:, kt, :],
                                wk[kt][:, n0:n1],
                                start=(kt == 0),
                                stop=(kt == KT - 1),
                            )
                        o_h = otail.tile(
                            [P, n1 - n0], mybir.dt.float32,
                            tag=f"ot{h}", name=f"ot{h}",
                        )
                        nc.vector.tensor_add(o_h[:], ph[:], b_sb[:, n0:n1])
                        engs[h].dma_start(out=o_d[mt, :, n0:n1], in_=o_h[:])
    nc.compile()
    return nc


def run(x, weight, fake_bias):
    global LAST_RESULT
    M, K = x.shape
    N = weight.shape[0]
    assert M % P == 0 and K % P == 0 and N % (N_CORES * P) == 0
    MT, KT, NLOC = M // P, K // P, N // N_CORES

    xb = np.asarray(x).astype(np.int8)
    x_tiles = np.ascontiguousarray(xb.reshape(MT, P, KT, P).transpose(0, 3, 2, 1))
    wb = np.asarray(weight).astype(np.int8)
    bias = np.asarray(fake_bias).astype(np.float32)

    in_maps = []
    for c in range(N_CORES):
        w_loc = wb[c * NLOC : (c + 1) * NLOC, :]  # [NLOC, K]
        w_tiles = np.ascontiguousarray(
            w_loc.T.reshape(KT, P, NLOC).transpose(1, 0, 2)
        )
        b_loc = np.ascontiguousarray(
            np.broadcast_to(bias[None, c * NLOC : (c + 1) * NLOC], (P, NLOC))
        )
        in_maps.append(
            {"x_tiles": x_tiles, "w_tiles": w_tiles, "bias_bcast": b_loc}
        )

    nc = build_program(MT, KT, NLOC)
    res = run_bass_kernel_spmd(
        nc, in_maps, list(range(N_CORES)), trace=TRACE, **TRACE_KWARGS
    )
    LAST_RESULT = res

    outs = [r["out_tiles"].reshape(M, NLOC) for r in res.results]
    return np.concatenate(outs, axis=1).astype(np.float32)


def kernel(x, weight, fake_bias):
    return run(x, weight, fake_bias)
